# revision 3
# baseline (speedup 1.0000x reference)
"""Trainium2 Bass kernel for nn_Decoder_17214228922493.

32-step LSTM decoder: B=64, H=1536, input=1024, applied to a constant input.
    xg = x @ W_ih.T + b_ih + b_hh                      (once)
    per step: gates = xg + h @ W_hh.T ; LSTM cell update ; emit h

Sharding: tensor-parallel over the gate dimension (8 cores x 768 gate
columns).  Each core keeps its slice of (h, c) plus a replicated copy of
h^T for the matmul contraction; after every step the 8 h^T slices are
re-assembled with an AllGather.  Gate columns are reordered per core to
[i | f | o | g] so one sigmoid covers i,f,o and one tanh covers g.

All compute is on-device; the host only slices/transposes weights into the
per-core layout and re-assembles the final [32, 64, 1536] output.
"""

import sys

if "/opt/trn_rl_repo" not in sys.path:
    sys.path.insert(0, "/opt/trn_rl_repo")

from contextlib import ExitStack

import numpy as np

import concourse.bass as bass
import concourse.mybir as mybir
import concourse.tile as tile
from concourse import bacc
from concourse import bass_utils
from concourse._compat import get_trn_type

F32 = mybir.dt.float32
R = 8          # cores
B = 64         # batch
H = 1536       # hidden
HL = H // R    # 192 per-core hidden slice
IN = 1024      # lstm input size
KA = 1152      # augmented input contraction (1024 + bias row, padded to 9*128)
NG = 4 * HL    # 768 gate columns per core
S = 32         # steps
NH = 384       # matmul moving free-dim (two groups of 384 = NG)

_CACHE = {}


def _build():
    nc = bacc.Bacc(
        get_trn_type() or "TRN2",
        target_bir_lowering=False,
        debug=False,
        num_devices=R,
    )

    xT = nc.dram_tensor("xT", [KA, B], F32, kind="ExternalInput")
    wih = nc.dram_tensor("wih", [KA, NG], F32, kind="ExternalInput")
    whh = nc.dram_tensor("whh", [H, NG], F32, kind="ExternalInput")
    h0T = nc.dram_tensor("h0T", [H, B], F32, kind="ExternalInput")
    c0 = nc.dram_tensor("c0", [B, HL], F32, kind="ExternalInput")
    iden = nc.dram_tensor("iden", [B, B], F32, kind="ExternalInput")
    out = nc.dram_tensor("out", [S, HL, B], F32, kind="ExternalOutput")

    bounces = [
        nc.dram_tensor(f"bounce{t}", [HL, B], F32, kind="Internal")
        for t in range(S - 1)
    ]
    gaths = [
        nc.dram_tensor(f"gath{t}", [H, B], F32, kind="Internal", addr_space="Shared")
        for t in range(S - 1)
    ]

    KHT = H // 128   # 12 k-tiles for the recurrent matmul
    KAT = KA // 128  # 9 k-tiles for the input matmul

    with ExitStack() as ctx:
        tc = ctx.enter_context(tile.TileContext(nc))
        wpool = ctx.enter_context(tc.tile_pool(name="w", bufs=1))
        cpool = ctx.enter_context(tc.tile_pool(name="cst", bufs=1))
        hpool = ctx.enter_context(tc.tile_pool(name="h", bufs=2))
        spool = ctx.enter_context(tc.tile_pool(name="s", bufs=3))
        gpool = ctx.enter_context(tc.tile_pool(name="g", bufs=2, space="PSUM"))
        tpool = ctx.enter_context(tc.tile_pool(name="t", bufs=2, space="PSUM"))

        whh_t = []
        for k in range(KHT):
            w = wpool.tile([128, NG], F32, tag=f"whh{k}")
            nc.sync.dma_start(w[:], whh[128 * k : 128 * (k + 1), :])
            whh_t.append(w)
        wih_t = []
        for k in range(KAT):
            w = wpool.tile([128, NG], F32, tag=f"wih{k}")
            nc.sync.dma_start(w[:], wih[128 * k : 128 * (k + 1), :])
            wih_t.append(w)
        x_t = []
        for k in range(KAT):
            xx = wpool.tile([128, B], F32, tag=f"x{k}")
            nc.sync.dma_start(xx[:], xT[128 * k : 128 * (k + 1), :])
            x_t.append(xx)
        iden_t = cpool.tile([B, B], F32, tag="iden")
        nc.sync.dma_start(iden_t[:], iden[:])

        # initial hidden state, transposed layout [H, B] in 3 chunks of 4 k-tiles
        def load_hT(src, tag_suffix):
            chunks = []
            for j in range(3):
                hc = hpool.tile([128, 4 * B], F32, tag=f"hc{j}")
                src_ap = src.rearrange("(k p) n -> p k n", p=128)[:, 4 * j : 4 * j + 4, :]
                nc.sync.dma_start(hc[:], src_ap)
                chunks.append(hc)
            return chunks

        def h_tile(chunks, k):
            return chunks[k // 4][:, B * (k % 4) : B * (k % 4 + 1)]

        h_chunks = load_hT(h0T, "init")

        c_t = spool.tile([B, HL], F32, tag="c")
        nc.sync.dma_start(c_t[:], c0[:])

        # xg = xT.T @ wih  (bias folded into augmented row 1024)
        # Two PSUM groups of 384 cols each: [i|f] and [o|g].  A single
        # matmul output must stay within one 2KB PSUM bank, so the 768
        # gate columns live in two separate 1-bank tiles.
        def gates_matmul(stat_tiles, w_tiles, nk, extra_first=None):
            halves = []
            for n in range(2):
                nsl = bass.ts(n, NH)
                p = gpool.tile([B, NH], F32, tag=f"g{n}")
                k0 = 0
                if extra_first is not None:
                    lhsT, rhs_full = extra_first
                    nc.tensor.matmul(
                        p[:], lhsT[:], rhs_full[:, nsl], start=True, stop=False
                    )
                    k0 = -1
                for k in range(nk):
                    nc.tensor.matmul(
                        p[:],
                        stat_tiles(k),
                        w_tiles[k][:, nsl],
                        start=(k == 0 and k0 == 0),
                        stop=(k == nk - 1),
                    )
                halves.append(p)
            return halves

        xgA, xgB = gates_matmul(lambda k: x_t[k][:], wih_t, KAT)
        xg_sb = cpool.tile([B, NG], F32, tag="xg")
        nc.vector.tensor_copy(xg_sb[:, 0:NH], xgA[:])
        nc.vector.tensor_copy(xg_sb[:, NH:NG], xgB[:])

        sig = mybir.ActivationFunctionType.Sigmoid
        tanh = mybir.ActivationFunctionType.Tanh

        for t in range(S):
            psA, psB = gates_matmul(
                lambda k: h_tile(h_chunks, k),
                whh_t,
                KHT,
                extra_first=(iden_t, xg_sb),
            )

            # psA = [i | f], psB = [o | g]
            s_if = spool.tile([B, 2 * HL], F32, tag="sif")
            nc.scalar.activation(s_if[:], psA[:], sig)
            s_o = spool.tile([B, HL], F32, tag="so")
            nc.scalar.activation(s_o[:], psB[:, 0:HL], sig)
            tg = spool.tile([B, HL], F32, tag="tg")
            nc.scalar.activation(tg[:], psB[:, HL : 2 * HL], tanh)

            m1 = spool.tile([B, HL], F32, tag="m1")
            nc.vector.tensor_mul(m1[:], s_if[:, HL : 2 * HL], c_t[:])  # f*c
            m2 = spool.tile([B, HL], F32, tag="m2")
            nc.vector.tensor_mul(m2[:], s_if[:, 0:HL], tg[:])  # i*g
            c_new = spool.tile([B, HL], F32, tag="c")
            nc.vector.tensor_add(c_new[:], m1[:], m2[:])
            c_t = c_new
            tc_sb = spool.tile([B, HL], F32, tag="tc")
            nc.scalar.activation(tc_sb[:], c_new[:], tanh)
            h_sb = spool.tile([B, HL], F32, tag="hsb")
            nc.vector.tensor_mul(h_sb[:], s_o[:], tc_sb[:])

            # transpose h [64, 192] -> [192, 64] via PE, then PSUM->SBUF
            pt0 = tpool.tile([128, B], F32, tag="ht0")
            nc.tensor.transpose(pt0[:], h_sb[:, 0:128], iden_t[:])
            pt1 = tpool.tile([B, B], F32, tag="ht1")
            nc.tensor.transpose(pt1[:], h_sb[:, 128:HL], iden_t[:])
            ht0 = spool.tile([128, B], F32, tag="ht0s")
            nc.vector.tensor_copy(ht0[:], pt0[:])
            ht1 = spool.tile([B, B], F32, tag="ht1s")
            nc.scalar.activation(ht1[:], pt1[:], mybir.ActivationFunctionType.Copy)

            nc.sync.dma_start(out[t, 0:128, :], ht0[:])
            nc.sync.dma_start(out[t, 128:HL, :], ht1[:])

            if t < S - 1:
                nc.sync.dma_start(bounces[t][0:128, :], ht0[:])
                nc.sync.dma_start(bounces[t][128:HL, :], ht1[:])
                nc.gpsimd.collective_compute(
                    "AllGather",
                    mybir.AluOpType.bypass,
                    replica_groups=[list(range(R))],
                    ins=[bounces[t][:]],
                    outs=[gaths[t][:]],
                )
                h_chunks = load_hT(gaths[t], f"s{t}")

    nc.compile()
    return nc


def _prep_inputs(sequence, hidden_state, cell_state, W_ih, W_hh, b_ih, b_hh):
    x = np.asarray(sequence, np.float32)[0]          # [64, 1024]
    h0 = np.asarray(hidden_state, np.float32)[0]     # [64, 1536]
    c0f = np.asarray(cell_state, np.float32)[0]
    W_ih = np.asarray(W_ih, np.float32)
    W_hh = np.asarray(W_hh, np.float32)
    b = (np.asarray(b_ih, np.float32) + np.asarray(b_hh, np.float32))

    xT = np.zeros((KA, B), np.float32)
    xT[:IN] = x.T
    xT[IN] = 1.0
    h0T = np.ascontiguousarray(h0.T)
    iden = np.eye(B, dtype=np.float32)

    in_maps = []
    for r in range(R):
        sl = np.arange(r * HL, (r + 1) * HL)
        sel = np.concatenate([sl, H + sl, 3 * H + sl, 2 * H + sl])  # i, f, o, g
        wa = np.zeros((KA, NG), np.float32)
        wa[:IN] = W_ih[sel].T
        wa[IN] = b[sel]
        in_maps.append(
            {
                "xT": xT,
                "wih": wa,
                "whh": np.ascontiguousarray(W_hh[sel].T),
                "h0T": h0T,
                "c0": np.ascontiguousarray(c0f[:, sl]),
                "iden": iden,
            }
        )
    return in_maps


def kernel(**inputs) -> np.ndarray:
    if "nc" not in _CACHE:
        _CACHE["nc"] = _build()
    nc = _CACHE["nc"]
    in_maps = _prep_inputs(**inputs)
    res = bass_utils.run_bass_kernel_spmd(nc, in_maps, core_ids=list(range(R)))
    preds = np.empty((S, B, H), np.float32)
    for r in range(R):
        o = res.results[r]["out"]  # [32, 192, 64]
        preds[:, :, r * HL : (r + 1) * HL] = np.transpose(o, (0, 2, 1))
    return preds


# revision 5
# speedup vs baseline: 1.4962x; 1.4962x over previous
"""Trainium2 Bass kernel for nn_Decoder_17214228922493.

32-step LSTM decoder: B=64, H=1536, input=1024, applied to a constant input.
    xg = x @ W_ih.T + b_ih + b_hh                      (once)
    per step: gates = xg + h @ W_hh.T ; LSTM cell update ; emit h

Sharding: tensor-parallel over the gate dimension (8 cores x 768 gate
columns).  Each core keeps its slice of (h, c) plus a replicated copy of
h^T for the matmul contraction; after every step the 8 h^T slices are
re-assembled with an AllGather.  Gate columns are reordered per core to
[i | f | o | g] so one sigmoid covers i,f,o and one tanh covers g.

All compute is on-device; the host only slices/transposes weights into the
per-core layout and re-assembles the final [32, 64, 1536] output.
"""

import sys

if "/opt/trn_rl_repo" not in sys.path:
    sys.path.insert(0, "/opt/trn_rl_repo")

from contextlib import ExitStack

import ml_dtypes
import numpy as np

import concourse.bass as bass
import concourse.mybir as mybir
import concourse.tile as tile
from concourse import bacc
from concourse import bass_utils
from concourse._compat import get_trn_type

F32 = mybir.dt.float32
BF16 = mybir.dt.bfloat16
R = 8          # cores
B = 64         # batch
H = 1536       # hidden
HL = H // R    # 192 per-core hidden slice
IN = 1024      # lstm input size
KA = 1152      # augmented input contraction (1024 + bias row, padded to 9*128)
NG = 4 * HL    # 768 gate columns per core
S = 32         # steps
NH = 384       # matmul moving free-dim (two groups of 384 = NG)

_CACHE = {}


def _build():
    nc = bacc.Bacc(
        get_trn_type() or "TRN2",
        target_bir_lowering=False,
        debug=False,
        num_devices=R,
    )

    xT = nc.dram_tensor("xT", [KA, B], BF16, kind="ExternalInput")
    wih = nc.dram_tensor("wih", [KA, NG], BF16, kind="ExternalInput")
    whh = nc.dram_tensor("whh", [H, NG], BF16, kind="ExternalInput")
    h0T = nc.dram_tensor("h0T", [H, B], BF16, kind="ExternalInput")
    c0 = nc.dram_tensor("c0", [B, HL], F32, kind="ExternalInput")
    iden = nc.dram_tensor("iden", [B, B], BF16, kind="ExternalInput")
    out = nc.dram_tensor("out", [S, HL, B], BF16, kind="ExternalOutput")

    bounces = [
        nc.dram_tensor(f"bounce{t}", [HL, B], BF16, kind="Internal")
        for t in range(S - 1)
    ]
    gaths = [
        nc.dram_tensor(f"gath{t}", [H, B], BF16, kind="Internal", addr_space="Shared")
        for t in range(S - 1)
    ]

    KHT = H // 128   # 12 k-tiles for the recurrent matmul
    KAT = KA // 128  # 9 k-tiles for the input matmul

    with ExitStack() as ctx:
        tc = ctx.enter_context(tile.TileContext(nc))
        wpool = ctx.enter_context(tc.tile_pool(name="w", bufs=1))
        cpool = ctx.enter_context(tc.tile_pool(name="cst", bufs=1))
        hpool = ctx.enter_context(tc.tile_pool(name="h", bufs=2))
        spool = ctx.enter_context(tc.tile_pool(name="s", bufs=3))
        gpool = ctx.enter_context(tc.tile_pool(name="g", bufs=2, space="PSUM"))
        tpool = ctx.enter_context(tc.tile_pool(name="t", bufs=2, space="PSUM"))

        whh_t = []
        for k in range(KHT):
            w = wpool.tile([128, NG], BF16, tag=f"whh{k}")
            nc.sync.dma_start(w[:], whh[128 * k : 128 * (k + 1), :])
            whh_t.append(w)
        wih_t = []
        for k in range(KAT):
            w = wpool.tile([128, NG], BF16, tag=f"wih{k}")
            nc.sync.dma_start(w[:], wih[128 * k : 128 * (k + 1), :])
            wih_t.append(w)
        x_t = []
        for k in range(KAT):
            xx = wpool.tile([128, B], BF16, tag=f"x{k}")
            nc.sync.dma_start(xx[:], xT[128 * k : 128 * (k + 1), :])
            x_t.append(xx)
        iden_t = cpool.tile([B, B], BF16, tag="iden")
        nc.sync.dma_start(iden_t[:], iden[:])

        # initial hidden state, transposed layout [H, B] in 3 chunks of 4 k-tiles
        def load_hT(src, tag_suffix):
            chunks = []
            for j in range(3):
                hc = hpool.tile([128, 4 * B], BF16, tag=f"hc{j}")
                src_ap = src.rearrange("(k p) n -> p k n", p=128)[:, 4 * j : 4 * j + 4, :]
                nc.sync.dma_start(hc[:], src_ap)
                chunks.append(hc)
            return chunks

        def h_tile(chunks, k):
            return chunks[k // 4][:, B * (k % 4) : B * (k % 4 + 1)]

        h_chunks = load_hT(h0T, "init")

        c_t = spool.tile([B, HL], F32, tag="c")
        nc.sync.dma_start(c_t[:], c0[:])

        # xg = xT.T @ wih  (bias folded into augmented row 1024)
        # Two PSUM groups of 384 cols each: [i|f] and [o|g].  A single
        # matmul output must stay within one 2KB PSUM bank, so the 768
        # gate columns live in two separate 1-bank tiles.
        def gates_matmul(stat_tiles, w_tiles, nk, extra_first=None):
            halves = []
            for n in range(2):
                nsl = bass.ts(n, NH)
                p = gpool.tile([B, NH], F32, tag=f"g{n}")
                k0 = 0
                if extra_first is not None:
                    lhsT, rhs_full = extra_first
                    nc.tensor.matmul(
                        p[:], lhsT[:], rhs_full[:, nsl], start=True, stop=False
                    )
                    k0 = -1
                for k in range(nk):
                    nc.tensor.matmul(
                        p[:],
                        stat_tiles(k),
                        w_tiles[k][:, nsl],
                        start=(k == 0 and k0 == 0),
                        stop=(k == nk - 1),
                    )
                halves.append(p)
            return halves

        xgA, xgB = gates_matmul(lambda k: x_t[k][:], wih_t, KAT)
        xg_sb = cpool.tile([B, NG], BF16, tag="xg")
        nc.vector.tensor_copy(xg_sb[:, 0:NH], xgA[:])
        nc.vector.tensor_copy(xg_sb[:, NH:NG], xgB[:])

        sig = mybir.ActivationFunctionType.Sigmoid
        tanh = mybir.ActivationFunctionType.Tanh

        for t in range(S):
            psA, psB = gates_matmul(
                lambda k: h_tile(h_chunks, k),
                whh_t,
                KHT,
                extra_first=(iden_t, xg_sb),
            )

            # psA = [i | f], psB = [o | g]
            s_if = spool.tile([B, 2 * HL], F32, tag="sif")
            nc.scalar.activation(s_if[:], psA[:], sig)
            s_o = spool.tile([B, HL], F32, tag="so")
            nc.scalar.activation(s_o[:], psB[:, 0:HL], sig)
            tg = spool.tile([B, HL], F32, tag="tg")
            nc.scalar.activation(tg[:], psB[:, HL : 2 * HL], tanh)

            m1 = spool.tile([B, HL], F32, tag="m1")
            nc.vector.tensor_mul(m1[:], s_if[:, HL : 2 * HL], c_t[:])  # f*c
            m2 = spool.tile([B, HL], F32, tag="m2")
            nc.vector.tensor_mul(m2[:], s_if[:, 0:HL], tg[:])  # i*g
            c_new = spool.tile([B, HL], F32, tag="c")
            nc.vector.tensor_add(c_new[:], m1[:], m2[:])
            c_t = c_new
            tc_sb = spool.tile([B, HL], F32, tag="tc")
            nc.scalar.activation(tc_sb[:], c_new[:], tanh)
            h_sb = spool.tile([B, HL], BF16, tag="hsb")
            nc.vector.tensor_mul(h_sb[:], s_o[:], tc_sb[:])

            # transpose h [64, 192] -> [192, 64] via PE, then PSUM->SBUF
            pt0 = tpool.tile([128, B], BF16, tag="ht0")
            nc.tensor.transpose(pt0[:], h_sb[:, 0:128], iden_t[:])
            pt1 = tpool.tile([B, B], BF16, tag="ht1")
            nc.tensor.transpose(pt1[:], h_sb[:, 128:HL], iden_t[:])
            ht0 = spool.tile([128, B], BF16, tag="ht0s")
            nc.vector.tensor_copy(ht0[:], pt0[:])
            ht1 = spool.tile([B, B], BF16, tag="ht1s")
            nc.scalar.activation(ht1[:], pt1[:], mybir.ActivationFunctionType.Copy)

            nc.sync.dma_start(out[t, 0:128, :], ht0[:])
            nc.sync.dma_start(out[t, 128:HL, :], ht1[:])

            if t < S - 1:
                nc.sync.dma_start(bounces[t][0:128, :], ht0[:])
                nc.sync.dma_start(bounces[t][128:HL, :], ht1[:])
                nc.gpsimd.collective_compute(
                    "AllGather",
                    mybir.AluOpType.bypass,
                    replica_groups=[list(range(R))],
                    ins=[bounces[t][:]],
                    outs=[gaths[t][:]],
                )
                h_chunks = load_hT(gaths[t], f"s{t}")

    nc.compile()
    return nc


def _prep_inputs(sequence, hidden_state, cell_state, W_ih, W_hh, b_ih, b_hh):
    x = np.asarray(sequence, np.float32)[0]          # [64, 1024]
    h0 = np.asarray(hidden_state, np.float32)[0]     # [64, 1536]
    c0f = np.asarray(cell_state, np.float32)[0]
    W_ih = np.asarray(W_ih, np.float32)
    W_hh = np.asarray(W_hh, np.float32)
    b = (np.asarray(b_ih, np.float32) + np.asarray(b_hh, np.float32))

    bf = ml_dtypes.bfloat16
    xT = np.zeros((KA, B), np.float32)
    xT[:IN] = x.T
    xT[IN] = 1.0
    xT = xT.astype(bf)
    h0T = np.ascontiguousarray(h0.T).astype(bf)
    iden = np.eye(B, dtype=bf)

    in_maps = []
    for r in range(R):
        sl = np.arange(r * HL, (r + 1) * HL)
        sel = np.concatenate([sl, H + sl, 3 * H + sl, 2 * H + sl])  # i, f, o, g
        wa = np.zeros((KA, NG), np.float32)
        wa[:IN] = W_ih[sel].T
        wa[IN] = b[sel]
        wa = wa.astype(bf)
        in_maps.append(
            {
                "xT": xT,
                "wih": wa,
                "whh": np.ascontiguousarray(W_hh[sel].T).astype(bf),
                "h0T": h0T,
                "c0": np.ascontiguousarray(c0f[:, sl]),
                "iden": iden,
            }
        )
    return in_maps


def kernel(**inputs) -> np.ndarray:
    if "nc" not in _CACHE:
        _CACHE["nc"] = _build()
    nc = _CACHE["nc"]
    in_maps = _prep_inputs(**inputs)
    res = bass_utils.run_bass_kernel_spmd(nc, in_maps, core_ids=list(range(R)))
    preds = np.empty((S, B, H), np.float32)
    for r in range(R):
        o = np.asarray(res.results[r]["out"], np.float32)  # [32, 192, 64]
        preds[:, :, r * HL : (r + 1) * HL] = np.transpose(o, (0, 2, 1))
    return preds
